# revision 26
# baseline (speedup 1.0000x reference)
"""Trainium2 Bass kernel for nn_Decoder_24541443129406.

Math: the reference's pdf/pdf_max cancels the normalization, so

    prob[n] = clip( sum_m exp( -0.5 * sum_d (pos[n,d]-mean[m,d])^2 / sigma[m,d] ), 0, 1 )

with pos = [ox, oy, dx, dy], sigma = [sx, sy, 1e-3, 1e-3],
sx = relu(l4)+0.01, sy = relu(l5)+0.01, mean = latents[:, :4].

The exponent is a quadratic form -> a K=8 matmul:
    e[n,m] = f[n] . w[m]
    f[n] = [dx^2+dy^2, 1, ox, oy, dx, dy, ox^2, oy^2]
    w[m] = [c7, c0, c1, c2, c3, c4, c5, c6]
      c1 = mx/sx, c2 = my/sy, c3 = 1000*mdx, c4 = 1000*mdy,
      c5 = -0.5/sx, c6 = -0.5/sy, c7 = -500,
      c0 = -0.5*(mx^2/sx + my^2/sy + 1000*(mdx^2+mdy^2))

fp32 matmuls are 4 cycles/row on the PE and float32r truncates, so the
K=8 fp32 matmul is emulated as one K=24 fp16 matmul with hi/lo split
operands stacked along K: e = h.H + l.H + h.L  (features f = h + l,
weights w = H + L, each half fp16; fp16 x fp16 products are exact in
fp32) — ~2^-22 relative accuracy at 1 cycle/row.

Per core (8 cores, data-parallel over rays): N_loc = 8192 rays, M = 512
gaussians. 16 super-tiles of 4 ray-blocks: 4x PE matmul -> PSUM
[128, 2048]; one ACT Exp pass -> fp16 [128, 2048] SBUF; per-block DVE
tensor_scalar with accum_out -> per-ray sums; clip; PE-transpose;
contiguous DMA out.
"""

import os
import sys
from contextlib import ExitStack

import numpy as np

for _p in ("/opt/trn_rl_repo", "/root/.axon_site/_ro/trn_rl_repo"):
    if os.path.isdir(_p) and _p not in sys.path:
        sys.path.insert(0, _p)

import concourse.bacc as bacc
import concourse.bass as bass
import concourse.mybir as mybir
import concourse.tile as tile
from concourse import bass_utils
from concourse.masks import make_identity

N_CORES = 8
N = 65536
M = 512
N_LOC = N // N_CORES  # 8192
NCHUNK = 32  # feature-build chunks (32-partition groups: verifier requires
# compute-op SBUF APs to start at partition 0/32/64/96)
CHUNK = N_LOC // NCHUNK  # 256
NBLK = N_LOC // 128  # 64 ray blocks of 128
NSUP = NBLK // 4  # 16 super-tiles of 4 blocks
SIGMA_EPS = 0.01
INV_SDIR = 1000.0  # 1/sigma_dir

F32 = mybir.dt.float32
F16 = mybir.dt.float16
ALU = mybir.AluOpType
ACTF = mybir.ActivationFunctionType

TRACE = False
LAST_PERF = None
_CACHED_NC = None


def build_kernel_body(nc, origins, directions, latents, prob):
    """origins/directions: [N_LOC, 2] f32 DRAM APs; latents [M, 6]; prob [N_LOC, 1]."""
    with tile.TileContext(nc) as tc, ExitStack() as ctx:
        singles = ctx.enter_context(tc.tile_pool(name="singles", bufs=1))
        scratch = ctx.enter_context(tc.tile_pool(name="scratch", bufs=3))

        # ---------------- input loads (parallel DMA queues) ----------------
        # Contiguous loads only; strided extraction happens on-chip where the
        # address generators make it free.
        raw_og = singles.tile([NCHUNK, 2 * CHUNK], F32)
        raw_dr = singles.tile([NCHUNK, 2 * CHUNK], F32)
        og_v = origins.rearrange("(i r) c -> i (r c)", i=NCHUNK)
        dr_v = directions.rearrange("(i r) c -> i (r c)", i=NCHUNK)
        # latents first: the weight path is the longest dependency chain
        lat32 = singles.tile([32, 96], F32)
        nc.scalar.dma_start(
            out=lat32, in_=latents.rearrange("(p j) f -> p (j f)", p=32)
        )
        nc.sync.dma_start(out=raw_dr[0:16, :], in_=dr_v[0:16, :])
        nc.scalar.dma_start(out=raw_dr[16:32, :], in_=dr_v[16:32, :])
        nc.gpsimd.dma_start(out=raw_og[0:16, :], in_=og_v[0:16, :])
        nc.sync.dma_start(out=raw_og[16:32, :], in_=og_v[16:32, :])

        # ---------------- gaussian weights ----------------
        # Entire weight prep runs 32-lane in the lat32 [32, 16-latents x 6]
        # layout (x/y live in the free dim -> no cross-partition hops).
        # lat32 view: value (p, j, f) = feature f of latent m = 16p + j.
        latv = lat32.rearrange("p (j f) -> p j f", f=6)

        def wtile(name):
            return singles.tile([32, 16], F32, name=name, tag=name)

        sx = wtile("sx")
        sy = wtile("sy")
        nc.vector.tensor_scalar(
            out=sx, in0=latv[:, :, 4], scalar1=0.0, scalar2=SIGMA_EPS,
            op0=ALU.max, op1=ALU.add,
        )
        nc.vector.tensor_scalar(
            out=sy, in0=latv[:, :, 5], scalar1=0.0, scalar2=SIGMA_EPS,
            op0=ALU.max, op1=ALU.add,
        )
        rx, ry = wtile("rx"), wtile("ry")
        rscr = wtile("rscr")
        nc.vector.reciprocal_approx_accurate(out=rx, in_=sx, scratch=rscr)
        nc.vector.reciprocal_approx_accurate(out=ry, in_=sy, scratch=rscr)

        c1, c2, c3, c4, c5, c6 = (wtile(f"c{i}") for i in range(1, 7))
        nc.vector.tensor_mul(out=c1, in0=latv[:, :, 0], in1=rx)
        nc.vector.tensor_mul(out=c2, in0=latv[:, :, 1], in1=ry)
        nc.vector.tensor_scalar_mul(out=c3, in0=latv[:, :, 2], scalar1=INV_SDIR)
        nc.vector.tensor_scalar_mul(out=c4, in0=latv[:, :, 3], scalar1=INV_SDIR)
        nc.vector.tensor_scalar_mul(out=c5, in0=rx, scalar1=-0.5)
        nc.vector.tensor_scalar_mul(out=c6, in0=ry, scalar1=-0.5)
        # c0 = -0.5*mx*c1 - 0.5*my*c2 - 500*mdx^2 - 500*mdy^2
        qx, qy, qdx, qdy = wtile("qx"), wtile("qy"), wtile("qdx"), wtile("qdy")
        nc.vector.scalar_tensor_tensor(
            out=qx, in0=latv[:, :, 0], scalar=-0.5, in1=c1,
            op0=ALU.mult, op1=ALU.mult,
        )
        nc.vector.scalar_tensor_tensor(
            out=qy, in0=latv[:, :, 1], scalar=-0.5, in1=c2,
            op0=ALU.mult, op1=ALU.mult,
        )
        nc.vector.scalar_tensor_tensor(
            out=qdx, in0=latv[:, :, 2], scalar=-0.5 * INV_SDIR, in1=latv[:, :, 2],
            op0=ALU.mult, op1=ALU.mult,
        )
        nc.vector.scalar_tensor_tensor(
            out=qdy, in0=latv[:, :, 3], scalar=-0.5 * INV_SDIR, in1=latv[:, :, 3],
            op0=ALU.mult, op1=ALU.mult,
        )
        c0 = wtile("c0")
        nc.vector.tensor_add(out=qx, in0=qx, in1=qy)
        nc.vector.tensor_add(out=qdx, in0=qdx, in1=qdy)
        nc.vector.tensor_add(out=c0, in0=qx, in1=qdx)

        # fp16 hi/lo split + assembly into the stacked weight tile
        # wgs [24, M]: rows 0-7 = H, 8-15 = H, 16-23 = L
        # (row order in each group: 0=c7, 1=c0, 2=c1 ... 7=c6).
        # [32, 16] f16 partition-major stream == m-order [1, 512] row.
        wgs = singles.tile([24, M], F16)
        c7h = singles.tile([1, M], F16)
        c7l = singles.tile([1, M], F16)
        nc.vector.memset(c7h, -0.5 * INV_SDIR)
        nc.vector.memset(c7l, 0.0)
        nc.sync.dma_start(out=wgs[0:1, :], in_=c7h)
        nc.scalar.dma_start(out=wgs[8:9, :], in_=c7h)
        nc.gpsimd.dma_start(out=wgs[16:17, :], in_=c7l)

        eng3 = [nc.sync, nc.scalar, nc.gpsimd]
        for r, piece in enumerate([c0, c1, c2, c3, c4, c5, c6], start=1):
            h = singles.tile([32, 16], F16, name=f"wh{r}", tag=f"wh{r}")
            lo = singles.tile([32, 16], F16, name=f"wl{r}", tag=f"wl{r}")
            nc.vector.tensor_copy(out=h, in_=piece)
            nc.vector.tensor_tensor(out=lo, in0=piece, in1=h, op=ALU.subtract)
            eng3[r % 3].dma_start(out=wgs[r : r + 1, :], in_=h)
            eng3[(r + 1) % 3].dma_start(out=wgs[8 + r : 9 + r, :], in_=h)
            eng3[(r + 2) % 3].dma_start(out=wgs[16 + r : 17 + r, :], in_=lo)

        # ---------------- feature tiles ----------------
        ox, oy = raw_og[:, 0::2], raw_og[:, 1::2]
        dx, dy = raw_dr[:, 0::2], raw_dr[:, 1::2]

        # Two f-major tiles, 32-partition feature groups.
        # featA groups: 0=dx^2+dy^2, 1=ones, 2=ox, 3=oy
        # featB groups: 4=dx, 5=dy, 6=ox^2, 7=oy^2
        featA = singles.tile([128, CHUNK], F32)
        featB = singles.tile([128, CHUNK], F32)
        s1 = singles.tile([NCHUNK, CHUNK], F32)

        nc.vector.tensor_mul(out=featA[0:32, :], in0=dx, in1=dx)
        nc.vector.tensor_mul(out=s1, in0=dy, in1=dy)
        nc.vector.tensor_add(out=featA[0:32, :], in0=featA[0:32, :], in1=s1)
        nc.vector.memset(featA[32:64, :], 1.0)
        nc.scalar.copy(out=featA[64:96, :], in_=ox)
        nc.scalar.copy(out=featA[96:128, :], in_=oy)
        nc.gpsimd.tensor_copy(out=featB[0:32, :], in_=dx)
        nc.gpsimd.tensor_copy(out=featB[32:64, :], in_=dy)
        nc.vector.tensor_mul(out=featB[64:96, :], in0=ox, in1=ox)
        nc.vector.tensor_mul(out=featB[96:128, :], in0=oy, in1=oy)

        # fp16 hi/lo split of the features
        hA = singles.tile([128, CHUNK], F16)
        lA = singles.tile([128, CHUNK], F16)
        hB = singles.tile([128, CHUNK], F16)
        lB = singles.tile([128, CHUNK], F16)
        nc.vector.tensor_copy(out=hA, in_=featA)
        nc.vector.tensor_tensor(out=lA, in0=featA, in1=hA, op=ALU.subtract)
        nc.vector.tensor_copy(out=hB, in_=featB)
        nc.gpsimd.tensor_tensor(out=lB, in0=featB, in1=hB, op=ALU.subtract)

        # permute to featcs [24, N_LOC] fp16: rows 0-7 = h, 8-15 = l,
        # 16-23 = h (K-stack [h; l; h] paired with wgs = [H; H; L]:
        # e = h.H + l.H + h.L).  One DMA per feature group: [32, CHUNK]
        # partition-major stream == C-order [1, N_LOC] row.
        featcs = singles.tile([24, N_LOC], F16)
        eng = [nc.sync, nc.scalar, nc.gpsimd, nc.sync]
        for f in range(4):
            hsrcA = hA[32 * f : 32 * (f + 1), :]
            hsrcB = hB[32 * f : 32 * (f + 1), :]
            eng[f].dma_start(out=featcs[f : f + 1, :], in_=hsrcA)
            eng[f].dma_start(out=featcs[4 + f : 5 + f, :], in_=hsrcB)
            eng[f].dma_start(out=featcs[16 + f : 17 + f, :], in_=hsrcA)
            eng[f].dma_start(out=featcs[20 + f : 21 + f, :], in_=hsrcB)
            eng[(f + 1) % 3].dma_start(
                out=featcs[8 + f : 9 + f, :], in_=lA[32 * f : 32 * (f + 1), :]
            )
            eng[(f + 2) % 3].dma_start(
                out=featcs[12 + f : 13 + f, :], in_=lB[32 * f : 32 * (f + 1), :]
            )

        # identity for the output transpose (one-time, overlaps setup)
        ident = singles.tile([128, 128], F32)
        make_identity(nc, ident)

        # ---------------- main loop ----------------
        # res (DVE accums) + res_act (ACT accums) are separate tiles so the
        # two engines' accumulator writes never serialize on tile deps;
        # both are zero-initialized and summed at the end.
        res = singles.tile([128, NBLK], F32)  # res[p, b] = sum_m exp(e), ray 128b+p
        res_act = singles.tile([128, NBLK], F32)
        nc.vector.memset(res, 0.0)
        nc.gpsimd.memset(res_act, 0.0)
        with tc.tile_pool(name="psum", bufs=2, space="PSUM") as psum:
            for s in range(NSUP):
                ps = psum.tile([128, 4 * M], F32, tag="ps")
                for j in range(4):
                    b = 4 * s + j
                    nc.tensor.matmul(
                        out=ps[:, M * j : M * (j + 1)],
                        lhsT=featcs[:, 128 * b : 128 * (b + 1)],
                        rhs=wgs,
                        start=True,
                        stop=True,
                    )
                # hybrid reduce: on odd super-tiles the 4th block's exp+sum
                # runs on ACT (accum_out), balancing ACT (~33us) vs DVE (~34us)
                ndve = 4 if s % 2 == 0 else 3
                ex = scratch.tile([128, 4 * M], F16, tag="ex")
                nc.scalar.activation(
                    out=ex[:, : ndve * M], in_=ps[:, : ndve * M], func=ACTF.Exp
                )
                if ndve == 3:
                    b3 = 4 * s + 3
                    nc.scalar.activation(
                        out=ex[:, 3 * M :],
                        in_=ps[:, 3 * M :],
                        func=ACTF.Exp,
                        accum_out=res_act[:, b3 : b3 + 1],
                    )
                for j in range(ndve):
                    b = 4 * s + j
                    sc = scratch.tile([128, M], F16, tag="sc")
                    nc.vector.tensor_scalar(
                        out=sc,
                        in0=ex[:, M * j : M * (j + 1)],
                        scalar1=0.0,
                        scalar2=0.0,
                        op0=ALU.add,
                        op1=ALU.add,
                        accum_out=res[:, b : b + 1],
                    )

        # merge the two accumulator tiles, then clip to [0, 1]
        nc.vector.tensor_add(out=res, in0=res, in1=res_act)
        nc.vector.tensor_scalar(
            out=res, in0=res, scalar1=0.0, scalar2=1.0, op0=ALU.max, op1=ALU.min
        )

        # transpose [128, NBLK] -> [NBLK, 128] so DRAM writes are contiguous
        with tc.tile_pool(name="psumt", bufs=1, space="PSUM") as psumt:
            pst = psumt.tile([NBLK, 128], F32)
            nc.tensor.transpose(out=pst, in_=res[:, :], identity=ident[:, :])
            rest = singles.tile([NBLK, 128], F32)
            nc.vector.tensor_copy(out=rest, in_=pst)
            nc.sync.dma_start(
                out=prob.rearrange("(b p) o -> b (p o)", b=NBLK), in_=rest
            )


def build_nc():
    nc = bacc.Bacc("TRN2", target_bir_lowering=False, debug=False)
    origins = nc.dram_tensor("origins", [N_LOC, 2], F32, kind="ExternalInput").ap()
    directions = nc.dram_tensor("directions", [N_LOC, 2], F32, kind="ExternalInput").ap()
    latents = nc.dram_tensor("latents", [M, 6], F32, kind="ExternalInput").ap()
    prob = nc.dram_tensor("prob", [N_LOC, 1], F32, kind="ExternalOutput").ap()
    build_kernel_body(nc, origins, directions, latents, prob)
    nc.compile()
    return nc


def kernel(origins: np.ndarray, directions: np.ndarray, latents: np.ndarray) -> np.ndarray:
    global _CACHED_NC, LAST_PERF
    assert origins.shape == (N, 2) and directions.shape == (N, 2)
    assert latents.shape == (M, 6)
    origins = np.ascontiguousarray(origins, dtype=np.float32)
    directions = np.ascontiguousarray(directions, dtype=np.float32)
    latents = np.ascontiguousarray(latents, dtype=np.float32)

    if _CACHED_NC is None:
        _CACHED_NC = build_nc()
    nc = _CACHED_NC

    in_maps = []
    for c in range(N_CORES):
        sl = slice(c * N_LOC, (c + 1) * N_LOC)
        in_maps.append(
            {
                "origins": origins[sl],
                "directions": directions[sl],
                "latents": latents,
            }
        )

    results = bass_utils.run_bass_kernel_spmd(
        nc,
        in_maps,
        core_ids=list(range(N_CORES)),
        trace=TRACE,
    )
    LAST_PERF = results
    out = np.concatenate([results.results[c]["prob"] for c in range(N_CORES)], axis=0)
    return out.astype(np.float32)


if __name__ == "__main__":
    rng = np.random.default_rng(0)
    o = rng.standard_normal((N, 2), dtype=np.float32)
    d = rng.standard_normal((N, 2), dtype=np.float32)
    l = rng.standard_normal((M, 6), dtype=np.float32)
    p = kernel(o, d, l)
    print(p.shape, p.dtype, p.min(), p.max())


# revision 29
# speedup vs baseline: 1.0521x; 1.0521x over previous
"""Trainium2 Bass kernel for nn_Decoder_24541443129406.

Math: the reference's pdf/pdf_max cancels the normalization, so

    prob[n] = clip( sum_m exp( -0.5 * sum_d (pos[n,d]-mean[m,d])^2 / sigma[m,d] ), 0, 1 )

with pos = [ox, oy, dx, dy], sigma = [sx, sy, 1e-3, 1e-3],
sx = relu(l4)+0.01, sy = relu(l5)+0.01, mean = latents[:, :4].

The exponent is a quadratic form -> a K=8 matmul:
    e[n,m] = f[n] . w[m]
    f[n] = [dx^2+dy^2, 1, ox, oy, dx, dy, ox^2, oy^2]
    w[m] = [c7, c0, c1, c2, c3, c4, c5, c6]
      c1 = mx/sx, c2 = my/sy, c3 = 1000*mdx, c4 = 1000*mdy,
      c5 = -0.5/sx, c6 = -0.5/sy, c7 = -500,
      c0 = -0.5*(mx^2/sx + my^2/sy + 1000*(mdx^2+mdy^2))

fp32 matmuls are 4 cycles/row on the PE and float32r truncates, so the
K=8 fp32 matmul is emulated as one K=24 fp16 matmul with hi/lo split
operands stacked along K: e = h.H + l.H + h.L  (features f = h + l,
weights w = H + L, each half fp16; fp16 x fp16 products are exact in
fp32) — ~2^-22 relative accuracy at 1 cycle/row.

Per core (8 cores, data-parallel over rays): N_loc = 8192 rays, M = 512
gaussians. 16 super-tiles of 4 ray-blocks: 4x PE matmul -> PSUM
[128, 2048]; one ACT Exp pass -> fp16 [128, 2048] SBUF; per-block DVE
tensor_scalar with accum_out -> per-ray sums; clip; PE-transpose;
contiguous DMA out.
"""

import os
import sys
from contextlib import ExitStack

import numpy as np

for _p in ("/opt/trn_rl_repo", "/root/.axon_site/_ro/trn_rl_repo"):
    if os.path.isdir(_p) and _p not in sys.path:
        sys.path.insert(0, _p)

import concourse.bacc as bacc
import concourse.bass as bass
import concourse.mybir as mybir
import concourse.tile as tile
from concourse import bass_utils
from concourse.masks import make_identity

N_CORES = 8
N = 65536
M = 512
N_LOC = N // N_CORES  # 8192
NCHUNK = 32  # feature-build chunks (32-partition groups: verifier requires
# compute-op SBUF APs to start at partition 0/32/64/96)
CHUNK = N_LOC // NCHUNK  # 256
NBLK = N_LOC // 128  # 64 ray blocks of 128
NSUP = NBLK // 4  # 16 super-tiles of 4 blocks
SIGMA_EPS = 0.01
INV_SDIR = 1000.0  # 1/sigma_dir

F32 = mybir.dt.float32
F16 = mybir.dt.float16
ALU = mybir.AluOpType
ACTF = mybir.ActivationFunctionType

TRACE = False
LAST_PERF = None
_CACHED_NC = None


def build_kernel_body(nc, origins, directions, latents, prob):
    """origins/directions: [N_LOC, 2] f32 DRAM APs; latents [M, 6]; prob [N_LOC, 1]."""
    with tile.TileContext(nc) as tc, ExitStack() as ctx:
        singles = ctx.enter_context(tc.tile_pool(name="singles", bufs=1))
        scratch = ctx.enter_context(tc.tile_pool(name="scratch", bufs=3))

        # ---------------- input loads (parallel DMA queues) ----------------
        # Contiguous loads only; strided extraction happens on-chip where the
        # address generators make it free.
        raw_og = singles.tile([NCHUNK, 2 * CHUNK], F32)
        raw_dr = singles.tile([NCHUNK, 2 * CHUNK], F32)
        og_v = origins.rearrange("(i r) c -> i (r c)", i=NCHUNK)
        dr_v = directions.rearrange("(i r) c -> i (r c)", i=NCHUNK)
        # latents first: the weight path is the longest dependency chain
        lat32 = singles.tile([32, 96], F32)
        nc.scalar.dma_start(
            out=lat32, in_=latents.rearrange("(p j) f -> p (j f)", p=32)
        )
        nc.sync.dma_start(out=raw_dr[0:16, :], in_=dr_v[0:16, :])
        nc.scalar.dma_start(out=raw_dr[16:32, :], in_=dr_v[16:32, :])
        nc.gpsimd.dma_start(out=raw_og[0:16, :], in_=og_v[0:16, :])
        nc.sync.dma_start(out=raw_og[16:32, :], in_=og_v[16:32, :])

        # ---------------- gaussian weights ----------------
        # Entire weight prep runs 32-lane in the lat32 [32, 16-latents x 6]
        # layout (x/y live in the free dim -> no cross-partition hops).
        # lat32 view: value (p, j, f) = feature f of latent m = 16p + j.
        latv = lat32.rearrange("p (j f) -> p j f", f=6)

        def wtile(name):
            return singles.tile([32, 16], F32, name=name, tag=name)

        sx = wtile("sx")
        sy = wtile("sy")
        nc.vector.tensor_scalar(
            out=sx, in0=latv[:, :, 4], scalar1=0.0, scalar2=SIGMA_EPS,
            op0=ALU.max, op1=ALU.add,
        )
        nc.vector.tensor_scalar(
            out=sy, in0=latv[:, :, 5], scalar1=0.0, scalar2=SIGMA_EPS,
            op0=ALU.max, op1=ALU.add,
        )
        rx, ry = wtile("rx"), wtile("ry")
        rscr = wtile("rscr")
        nc.vector.reciprocal_approx_accurate(out=rx, in_=sx, scratch=rscr)
        nc.vector.reciprocal_approx_accurate(out=ry, in_=sy, scratch=rscr)

        c1, c2, c3, c4, c5, c6 = (wtile(f"c{i}") for i in range(1, 7))
        nc.vector.tensor_mul(out=c1, in0=latv[:, :, 0], in1=rx)
        nc.vector.tensor_mul(out=c2, in0=latv[:, :, 1], in1=ry)
        nc.vector.tensor_scalar_mul(out=c3, in0=latv[:, :, 2], scalar1=INV_SDIR)
        nc.vector.tensor_scalar_mul(out=c4, in0=latv[:, :, 3], scalar1=INV_SDIR)
        nc.vector.tensor_scalar_mul(out=c5, in0=rx, scalar1=-0.5)
        nc.vector.tensor_scalar_mul(out=c6, in0=ry, scalar1=-0.5)
        # c0 = -0.5*mx*c1 - 0.5*my*c2 - 500*mdx^2 - 500*mdy^2
        qx, qy, qdx, qdy = wtile("qx"), wtile("qy"), wtile("qdx"), wtile("qdy")
        nc.vector.scalar_tensor_tensor(
            out=qx, in0=latv[:, :, 0], scalar=-0.5, in1=c1,
            op0=ALU.mult, op1=ALU.mult,
        )
        nc.vector.scalar_tensor_tensor(
            out=qy, in0=latv[:, :, 1], scalar=-0.5, in1=c2,
            op0=ALU.mult, op1=ALU.mult,
        )
        nc.vector.scalar_tensor_tensor(
            out=qdx, in0=latv[:, :, 2], scalar=-0.5 * INV_SDIR, in1=latv[:, :, 2],
            op0=ALU.mult, op1=ALU.mult,
        )
        nc.vector.scalar_tensor_tensor(
            out=qdy, in0=latv[:, :, 3], scalar=-0.5 * INV_SDIR, in1=latv[:, :, 3],
            op0=ALU.mult, op1=ALU.mult,
        )
        c0 = wtile("c0")
        nc.vector.tensor_add(out=qx, in0=qx, in1=qy)
        nc.vector.tensor_add(out=qdx, in0=qdx, in1=qdy)
        nc.vector.tensor_add(out=c0, in0=qx, in1=qdx)

        # fp16 hi/lo split + assembly into the stacked weight tile
        # wgs [24, M]: rows 0-7 = H, 8-15 = H, 16-23 = L
        # (row order in each group: 0=c7, 1=c0, 2=c1 ... 7=c6).
        # [32, 16] f16 partition-major stream == m-order [1, 512] row.
        wgs = singles.tile([24, M], F16)
        c7h = singles.tile([1, M], F16)
        c7l = singles.tile([1, M], F16)
        nc.vector.memset(c7h, -0.5 * INV_SDIR)
        nc.vector.memset(c7l, 0.0)
        nc.sync.dma_start(out=wgs[0:1, :], in_=c7h)
        nc.scalar.dma_start(out=wgs[8:9, :], in_=c7h)
        nc.gpsimd.dma_start(out=wgs[16:17, :], in_=c7l)

        eng3 = [nc.sync, nc.scalar, nc.gpsimd]
        for r, piece in enumerate([c0, c1, c2, c3, c4, c5, c6], start=1):
            h = singles.tile([32, 16], F16, name=f"wh{r}", tag=f"wh{r}")
            lo = singles.tile([32, 16], F16, name=f"wl{r}", tag=f"wl{r}")
            nc.vector.tensor_copy(out=h, in_=piece)
            nc.vector.tensor_tensor(out=lo, in0=piece, in1=h, op=ALU.subtract)
            eng3[r % 3].dma_start(out=wgs[r : r + 1, :], in_=h)
            eng3[(r + 1) % 3].dma_start(out=wgs[8 + r : 9 + r, :], in_=h)
            eng3[(r + 2) % 3].dma_start(out=wgs[16 + r : 17 + r, :], in_=lo)

        # ---------------- feature tiles ----------------
        ox, oy = raw_og[:, 0::2], raw_og[:, 1::2]
        dx, dy = raw_dr[:, 0::2], raw_dr[:, 1::2]

        # Two f-major tiles, 32-partition feature groups.
        # featA groups: 0=dx^2+dy^2, 1=ones, 2=ox, 3=oy
        # featB groups: 4=dx, 5=dy, 6=ox^2, 7=oy^2
        featA = singles.tile([128, CHUNK], F32)
        featB = singles.tile([128, CHUNK], F32)
        s1 = singles.tile([NCHUNK, CHUNK], F32)

        nc.vector.tensor_mul(out=featA[0:32, :], in0=dx, in1=dx)
        nc.vector.tensor_mul(out=s1, in0=dy, in1=dy)
        nc.vector.tensor_add(out=featA[0:32, :], in0=featA[0:32, :], in1=s1)
        nc.vector.memset(featA[32:64, :], 1.0)
        nc.scalar.copy(out=featA[64:96, :], in_=ox)
        nc.scalar.copy(out=featA[96:128, :], in_=oy)
        nc.gpsimd.tensor_copy(out=featB[0:32, :], in_=dx)
        nc.gpsimd.tensor_copy(out=featB[32:64, :], in_=dy)
        nc.vector.tensor_mul(out=featB[64:96, :], in0=ox, in1=ox)
        nc.vector.tensor_mul(out=featB[96:128, :], in0=oy, in1=oy)

        # fp16 hi/lo split of the features
        hA = singles.tile([128, CHUNK], F16)
        lA = singles.tile([128, CHUNK], F16)
        hB = singles.tile([128, CHUNK], F16)
        lB = singles.tile([128, CHUNK], F16)
        nc.vector.tensor_copy(out=hA, in_=featA)
        nc.vector.tensor_tensor(out=lA, in0=featA, in1=hA, op=ALU.subtract)
        nc.vector.tensor_copy(out=hB, in_=featB)
        nc.gpsimd.tensor_tensor(out=lB, in0=featB, in1=hB, op=ALU.subtract)

        # permute to featcs [24, N_LOC] fp16: rows 0-7 = h, 8-15 = l,
        # 16-23 = h (K-stack [h; l; h] paired with wgs = [H; H; L]:
        # e = h.H + l.H + h.L).  One DMA per feature group: [32, CHUNK]
        # partition-major stream == C-order [1, N_LOC] row.
        featcs = singles.tile([24, N_LOC], F16)
        eng = [nc.sync, nc.scalar, nc.gpsimd, nc.sync]
        for f in range(4):
            hsrcA = hA[32 * f : 32 * (f + 1), :]
            hsrcB = hB[32 * f : 32 * (f + 1), :]
            eng[f].dma_start(out=featcs[f : f + 1, :], in_=hsrcA)
            eng[f].dma_start(out=featcs[4 + f : 5 + f, :], in_=hsrcB)
            eng[f].dma_start(out=featcs[16 + f : 17 + f, :], in_=hsrcA)
            eng[f].dma_start(out=featcs[20 + f : 21 + f, :], in_=hsrcB)
            eng[(f + 1) % 3].dma_start(
                out=featcs[8 + f : 9 + f, :], in_=lA[32 * f : 32 * (f + 1), :]
            )
            eng[(f + 2) % 3].dma_start(
                out=featcs[12 + f : 13 + f, :], in_=lB[32 * f : 32 * (f + 1), :]
            )

        # identity for the output transpose (one-time, overlaps setup)
        ident = singles.tile([128, 128], F32)
        make_identity(nc, ident)

        # ---------------- main loop ----------------
        res = singles.tile([128, NBLK], F32)  # res[p, b] = sum_m exp(e), ray 128b+p
        with tc.tile_pool(name="psum", bufs=2, space="PSUM") as psum:
            for s in range(NSUP):
                ps = psum.tile([128, 4 * M], F32, tag="ps")
                for j in range(4):
                    b = 4 * s + j
                    nc.tensor.matmul(
                        out=ps[:, M * j : M * (j + 1)],
                        lhsT=featcs[:, 128 * b : 128 * (b + 1)],
                        rhs=wgs,
                        start=True,
                        stop=True,
                    )
                ex = scratch.tile([128, 4 * M], F16, tag="ex")
                nc.scalar.activation(out=ex, in_=ps, func=ACTF.Exp)
                for j in range(4):
                    b = 4 * s + j
                    sc = scratch.tile([128, M], F16, tag="sc")
                    nc.vector.tensor_scalar(
                        out=sc,
                        in0=ex[:, M * j : M * (j + 1)],
                        scalar1=0.0,
                        scalar2=0.0,
                        op0=ALU.add,
                        op1=ALU.add,
                        accum_out=res[:, b : b + 1],
                    )

        # clip to [0, 1]
        nc.vector.tensor_scalar(
            out=res, in0=res, scalar1=0.0, scalar2=1.0, op0=ALU.max, op1=ALU.min
        )

        # transpose [128, NBLK] -> [NBLK, 128] so DRAM writes are contiguous
        with tc.tile_pool(name="psumt", bufs=1, space="PSUM") as psumt:
            pst = psumt.tile([NBLK, 128], F32)
            nc.tensor.transpose(out=pst, in_=res[:, :], identity=ident[:, :])
            rest = singles.tile([NBLK, 128], F32)
            nc.vector.tensor_copy(out=rest, in_=pst)
            nc.sync.dma_start(
                out=prob.rearrange("(b p) o -> b (p o)", b=NBLK), in_=rest
            )


def build_nc():
    nc = bacc.Bacc("TRN2", target_bir_lowering=False, debug=False)
    origins = nc.dram_tensor("origins", [N_LOC, 2], F32, kind="ExternalInput").ap()
    directions = nc.dram_tensor("directions", [N_LOC, 2], F32, kind="ExternalInput").ap()
    latents = nc.dram_tensor("latents", [M, 6], F32, kind="ExternalInput").ap()
    prob = nc.dram_tensor("prob", [N_LOC, 1], F32, kind="ExternalOutput").ap()
    build_kernel_body(nc, origins, directions, latents, prob)
    nc.compile()
    return nc


def kernel(origins: np.ndarray, directions: np.ndarray, latents: np.ndarray) -> np.ndarray:
    global _CACHED_NC, LAST_PERF
    assert origins.shape == (N, 2) and directions.shape == (N, 2)
    assert latents.shape == (M, 6)
    origins = np.ascontiguousarray(origins, dtype=np.float32)
    directions = np.ascontiguousarray(directions, dtype=np.float32)
    latents = np.ascontiguousarray(latents, dtype=np.float32)

    if _CACHED_NC is None:
        _CACHED_NC = build_nc()
    nc = _CACHED_NC

    in_maps = []
    for c in range(N_CORES):
        sl = slice(c * N_LOC, (c + 1) * N_LOC)
        in_maps.append(
            {
                "origins": origins[sl],
                "directions": directions[sl],
                "latents": latents,
            }
        )

    results = bass_utils.run_bass_kernel_spmd(
        nc,
        in_maps,
        core_ids=list(range(N_CORES)),
        trace=TRACE,
    )
    LAST_PERF = results
    out = np.concatenate([results.results[c]["prob"] for c in range(N_CORES)], axis=0)
    return out.astype(np.float32)


if __name__ == "__main__":
    rng = np.random.default_rng(0)
    o = rng.standard_normal((N, 2), dtype=np.float32)
    d = rng.standard_normal((N, 2), dtype=np.float32)
    l = rng.standard_normal((M, 6), dtype=np.float32)
    p = kernel(o, d, l)
    print(p.shape, p.dtype, p.min(), p.max())


# revision 31
# speedup vs baseline: 1.0653x; 1.0125x over previous
"""Trainium2 Bass kernel for nn_Decoder_24541443129406.

Math: the reference's pdf/pdf_max cancels the normalization, so

    prob[n] = clip( sum_m exp( -0.5 * sum_d (pos[n,d]-mean[m,d])^2 / sigma[m,d] ), 0, 1 )

with pos = [ox, oy, dx, dy], sigma = [sx, sy, 1e-3, 1e-3],
sx = relu(l4)+0.01, sy = relu(l5)+0.01, mean = latents[:, :4].

The exponent is a quadratic form -> a K=8 matmul:
    e[n,m] = f[n] . w[m]
    f[n] = [dx^2+dy^2, 1, ox, oy, dx, dy, ox^2, oy^2]
    w[m] = [c7, c0, c1, c2, c3, c4, c5, c6]
      c1 = mx/sx, c2 = my/sy, c3 = 1000*mdx, c4 = 1000*mdy,
      c5 = -0.5/sx, c6 = -0.5/sy, c7 = -500,
      c0 = -0.5*(mx^2/sx + my^2/sy + 1000*(mdx^2+mdy^2))

fp32 matmuls are 4 cycles/row on the PE and float32r truncates, so the
K=8 fp32 matmul is emulated as one K=24 fp16 matmul with hi/lo split
operands stacked along K: e = h.H + l.H + h.L  (features f = h + l,
weights w = H + L, each half fp16; fp16 x fp16 products are exact in
fp32) — ~2^-22 relative accuracy at 1 cycle/row.

Per core (8 cores, data-parallel over rays): N_loc = 8192 rays, M = 512
gaussians. 16 super-tiles of 4 ray-blocks: 4x PE matmul -> PSUM
[128, 2048]; one ACT Exp pass -> fp16 [128, 2048] SBUF; per-block DVE
tensor_scalar with accum_out -> per-ray sums; clip; PE-transpose;
contiguous DMA out.
"""

import os
import sys
from contextlib import ExitStack

import numpy as np

for _p in ("/opt/trn_rl_repo", "/root/.axon_site/_ro/trn_rl_repo"):
    if os.path.isdir(_p) and _p not in sys.path:
        sys.path.insert(0, _p)

import concourse.bacc as bacc
import concourse.bass as bass
import concourse.mybir as mybir
import concourse.tile as tile
from concourse import bass_utils
from concourse.masks import make_identity

N_CORES = 8
N = 65536
M = 512
N_LOC = N // N_CORES  # 8192
NCHUNK = 32  # feature-build chunks (32-partition groups: verifier requires
# compute-op SBUF APs to start at partition 0/32/64/96)
CHUNK = N_LOC // NCHUNK  # 256
NBLK = N_LOC // 128  # 64 ray blocks of 128
NSUP = NBLK // 4  # 16 super-tiles of 4 blocks
SIGMA_EPS = 0.01
INV_SDIR = 1000.0  # 1/sigma_dir

F32 = mybir.dt.float32
F16 = mybir.dt.float16
ALU = mybir.AluOpType
ACTF = mybir.ActivationFunctionType

TRACE = False
LAST_PERF = None
_CACHED_NC = None


def build_kernel_body(nc, origins, directions, latents, prob):
    """origins/directions: [N_LOC, 2] f32 DRAM APs; latents [M, 6]; prob [N_LOC, 1]."""
    with tile.TileContext(nc) as tc, ExitStack() as ctx:
        singles = ctx.enter_context(tc.tile_pool(name="singles", bufs=1))
        scratch = ctx.enter_context(tc.tile_pool(name="scratch", bufs=4))

        # ---------------- input loads (parallel DMA queues) ----------------
        # Contiguous loads only; strided extraction happens on-chip where the
        # address generators make it free.
        raw_og = singles.tile([NCHUNK, 2 * CHUNK], F32)
        raw_dr = singles.tile([NCHUNK, 2 * CHUNK], F32)
        og_v = origins.rearrange("(i r) c -> i (r c)", i=NCHUNK)
        dr_v = directions.rearrange("(i r) c -> i (r c)", i=NCHUNK)
        # latents first: the weight path is the longest dependency chain
        lat32 = singles.tile([32, 96], F32)
        nc.scalar.dma_start(
            out=lat32, in_=latents.rearrange("(p j) f -> p (j f)", p=32)
        )
        nc.sync.dma_start(out=raw_dr[0:8, :], in_=dr_v[0:8, :])
        nc.gpsimd.dma_start(out=raw_dr[8:16, :], in_=dr_v[8:16, :])
        nc.scalar.dma_start(out=raw_dr[16:24, :], in_=dr_v[16:24, :])
        nc.sync.dma_start(out=raw_dr[24:32, :], in_=dr_v[24:32, :])
        nc.gpsimd.dma_start(out=raw_og[0:8, :], in_=og_v[0:8, :])
        nc.scalar.dma_start(out=raw_og[8:16, :], in_=og_v[8:16, :])
        nc.sync.dma_start(out=raw_og[16:24, :], in_=og_v[16:24, :])
        nc.gpsimd.dma_start(out=raw_og[24:32, :], in_=og_v[24:32, :])

        # ---------------- gaussian weights ----------------
        # Entire weight prep runs 32-lane in the lat32 [32, 16-latents x 6]
        # layout (x/y live in the free dim -> no cross-partition hops).
        # lat32 view: value (p, j, f) = feature f of latent m = 16p + j.
        latv = lat32.rearrange("p (j f) -> p j f", f=6)

        def wtile(name):
            return singles.tile([32, 16], F32, name=name, tag=name)

        sx = wtile("sx")
        sy = wtile("sy")
        nc.vector.tensor_scalar(
            out=sx, in0=latv[:, :, 4], scalar1=0.0, scalar2=SIGMA_EPS,
            op0=ALU.max, op1=ALU.add,
        )
        nc.vector.tensor_scalar(
            out=sy, in0=latv[:, :, 5], scalar1=0.0, scalar2=SIGMA_EPS,
            op0=ALU.max, op1=ALU.add,
        )
        rx, ry = wtile("rx"), wtile("ry")
        rscr = wtile("rscr")
        nc.vector.reciprocal_approx_accurate(out=rx, in_=sx, scratch=rscr)
        nc.vector.reciprocal_approx_accurate(out=ry, in_=sy, scratch=rscr)

        c1, c2, c3, c4, c5, c6 = (wtile(f"c{i}") for i in range(1, 7))
        nc.vector.tensor_mul(out=c1, in0=latv[:, :, 0], in1=rx)
        nc.vector.tensor_mul(out=c2, in0=latv[:, :, 1], in1=ry)
        nc.vector.tensor_scalar_mul(out=c3, in0=latv[:, :, 2], scalar1=INV_SDIR)
        nc.vector.tensor_scalar_mul(out=c4, in0=latv[:, :, 3], scalar1=INV_SDIR)
        nc.vector.tensor_scalar_mul(out=c5, in0=rx, scalar1=-0.5)
        nc.vector.tensor_scalar_mul(out=c6, in0=ry, scalar1=-0.5)
        # c0 = -0.5*mx*c1 - 0.5*my*c2 - 500*mdx^2 - 500*mdy^2
        qx, qy, qdx, qdy = wtile("qx"), wtile("qy"), wtile("qdx"), wtile("qdy")
        nc.vector.scalar_tensor_tensor(
            out=qx, in0=latv[:, :, 0], scalar=-0.5, in1=c1,
            op0=ALU.mult, op1=ALU.mult,
        )
        nc.vector.scalar_tensor_tensor(
            out=qy, in0=latv[:, :, 1], scalar=-0.5, in1=c2,
            op0=ALU.mult, op1=ALU.mult,
        )
        nc.vector.scalar_tensor_tensor(
            out=qdx, in0=latv[:, :, 2], scalar=-0.5 * INV_SDIR, in1=latv[:, :, 2],
            op0=ALU.mult, op1=ALU.mult,
        )
        nc.vector.scalar_tensor_tensor(
            out=qdy, in0=latv[:, :, 3], scalar=-0.5 * INV_SDIR, in1=latv[:, :, 3],
            op0=ALU.mult, op1=ALU.mult,
        )
        c0 = wtile("c0")
        nc.vector.tensor_add(out=qx, in0=qx, in1=qy)
        nc.vector.tensor_add(out=qdx, in0=qdx, in1=qdy)
        nc.vector.tensor_add(out=c0, in0=qx, in1=qdx)

        # fp16 hi/lo split + assembly into the stacked weight tile
        # wgs [24, M]: rows 0-7 = H, 8-15 = H, 16-23 = L
        # (row order in each group: 0=c7, 1=c0, 2=c1 ... 7=c6).
        # [32, 16] f16 partition-major stream == m-order [1, 512] row.
        wgs = singles.tile([24, M], F16)
        c7h = singles.tile([1, M], F16)
        c7l = singles.tile([1, M], F16)
        nc.vector.memset(c7h, -0.5 * INV_SDIR)
        nc.vector.memset(c7l, 0.0)
        nc.sync.dma_start(out=wgs[0:1, :], in_=c7h)
        nc.scalar.dma_start(out=wgs[8:9, :], in_=c7h)
        nc.gpsimd.dma_start(out=wgs[16:17, :], in_=c7l)

        eng3 = [nc.sync, nc.scalar, nc.gpsimd]
        for r, piece in enumerate([c0, c1, c2, c3, c4, c5, c6], start=1):
            h = singles.tile([32, 16], F16, name=f"wh{r}", tag=f"wh{r}")
            lo = singles.tile([32, 16], F16, name=f"wl{r}", tag=f"wl{r}")
            nc.vector.tensor_copy(out=h, in_=piece)
            nc.vector.tensor_tensor(out=lo, in0=piece, in1=h, op=ALU.subtract)
            eng3[r % 3].dma_start(out=wgs[r : r + 1, :], in_=h)
            eng3[(r + 1) % 3].dma_start(out=wgs[8 + r : 9 + r, :], in_=h)
            eng3[(r + 2) % 3].dma_start(out=wgs[16 + r : 17 + r, :], in_=lo)

        # ---------------- feature tiles ----------------
        ox, oy = raw_og[:, 0::2], raw_og[:, 1::2]
        dx, dy = raw_dr[:, 0::2], raw_dr[:, 1::2]

        # Two f-major tiles, 32-partition feature groups.
        # featA groups: 0=dx^2+dy^2, 1=ones, 2=ox, 3=oy
        # featB groups: 4=dx, 5=dy, 6=ox^2, 7=oy^2
        featA = singles.tile([128, CHUNK], F32)
        featB = singles.tile([128, CHUNK], F32)
        s1 = singles.tile([NCHUNK, CHUNK], F32)

        nc.vector.tensor_mul(out=featA[0:32, :], in0=dx, in1=dx)
        nc.vector.tensor_mul(out=s1, in0=dy, in1=dy)
        nc.vector.tensor_add(out=featA[0:32, :], in0=featA[0:32, :], in1=s1)
        nc.vector.memset(featA[32:64, :], 1.0)
        nc.scalar.copy(out=featA[64:96, :], in_=ox)
        nc.scalar.copy(out=featA[96:128, :], in_=oy)
        nc.gpsimd.tensor_copy(out=featB[0:32, :], in_=dx)
        nc.gpsimd.tensor_copy(out=featB[32:64, :], in_=dy)
        nc.vector.tensor_mul(out=featB[64:96, :], in0=ox, in1=ox)
        nc.vector.tensor_mul(out=featB[96:128, :], in0=oy, in1=oy)

        # fp16 hi/lo split of the features
        hA = singles.tile([128, CHUNK], F16)
        lA = singles.tile([128, CHUNK], F16)
        hB = singles.tile([128, CHUNK], F16)
        lB = singles.tile([128, CHUNK], F16)
        nc.vector.tensor_copy(out=hA, in_=featA)
        nc.vector.tensor_tensor(out=lA, in0=featA, in1=hA, op=ALU.subtract)
        nc.vector.tensor_copy(out=hB, in_=featB)
        nc.gpsimd.tensor_tensor(out=lB, in0=featB, in1=hB, op=ALU.subtract)

        # permute to featcs [24, N_LOC] fp16: rows 0-7 = h, 8-15 = l,
        # 16-23 = h (K-stack [h; l; h] paired with wgs = [H; H; L]:
        # e = h.H + l.H + h.L).  One DMA per feature group: [32, CHUNK]
        # partition-major stream == C-order [1, N_LOC] row.
        featcs = singles.tile([24, N_LOC], F16)
        eng = [nc.sync, nc.scalar, nc.gpsimd, nc.sync]
        for f in range(4):
            hsrcA = hA[32 * f : 32 * (f + 1), :]
            hsrcB = hB[32 * f : 32 * (f + 1), :]
            eng[f].dma_start(out=featcs[f : f + 1, :], in_=hsrcA)
            eng[f].dma_start(out=featcs[4 + f : 5 + f, :], in_=hsrcB)
            eng[f].dma_start(out=featcs[16 + f : 17 + f, :], in_=hsrcA)
            eng[f].dma_start(out=featcs[20 + f : 21 + f, :], in_=hsrcB)
            eng[(f + 1) % 3].dma_start(
                out=featcs[8 + f : 9 + f, :], in_=lA[32 * f : 32 * (f + 1), :]
            )
            eng[(f + 2) % 3].dma_start(
                out=featcs[12 + f : 13 + f, :], in_=lB[32 * f : 32 * (f + 1), :]
            )

        # identity for the output transpose (one-time, overlaps setup)
        ident = singles.tile([128, 128], F32)
        make_identity(nc, ident)

        # ---------------- main loop ----------------
        res = singles.tile([128, NBLK], F32)  # res[p, b] = sum_m exp(e), ray 128b+p
        with tc.tile_pool(name="psum", bufs=2, space="PSUM") as psum:
            for s in range(NSUP):
                ps = psum.tile([128, 4 * M], F32, tag="ps")
                for j in range(4):
                    b = 4 * s + j
                    nc.tensor.matmul(
                        out=ps[:, M * j : M * (j + 1)],
                        lhsT=featcs[:, 128 * b : 128 * (b + 1)],
                        rhs=wgs,
                        start=True,
                        stop=True,
                    )
                ex = scratch.tile([128, 4 * M], F16, tag="ex")
                nc.scalar.activation(out=ex, in_=ps, func=ACTF.Exp)
                for j in range(4):
                    b = 4 * s + j
                    sc = scratch.tile([128, M], F16, tag="sc")
                    nc.vector.tensor_scalar(
                        out=sc,
                        in0=ex[:, M * j : M * (j + 1)],
                        scalar1=0.0,
                        scalar2=0.0,
                        op0=ALU.add,
                        op1=ALU.add,
                        accum_out=res[:, b : b + 1],
                    )

        # clip to [0, 1]
        nc.vector.tensor_scalar(
            out=res, in0=res, scalar1=0.0, scalar2=1.0, op0=ALU.max, op1=ALU.min
        )

        # transpose [128, NBLK] -> [NBLK, 128] so DRAM writes are contiguous
        with tc.tile_pool(name="psumt", bufs=1, space="PSUM") as psumt:
            pst = psumt.tile([NBLK, 128], F32)
            nc.tensor.transpose(out=pst, in_=res[:, :], identity=ident[:, :])
            rest = singles.tile([NBLK, 128], F32)
            nc.vector.tensor_copy(out=rest, in_=pst)
            nc.sync.dma_start(
                out=prob.rearrange("(b p) o -> b (p o)", b=NBLK), in_=rest
            )


def build_nc():
    nc = bacc.Bacc("TRN2", target_bir_lowering=False, debug=False)
    origins = nc.dram_tensor("origins", [N_LOC, 2], F32, kind="ExternalInput").ap()
    directions = nc.dram_tensor("directions", [N_LOC, 2], F32, kind="ExternalInput").ap()
    latents = nc.dram_tensor("latents", [M, 6], F32, kind="ExternalInput").ap()
    prob = nc.dram_tensor("prob", [N_LOC, 1], F32, kind="ExternalOutput").ap()
    build_kernel_body(nc, origins, directions, latents, prob)
    nc.compile()
    return nc


def kernel(origins: np.ndarray, directions: np.ndarray, latents: np.ndarray) -> np.ndarray:
    global _CACHED_NC, LAST_PERF
    assert origins.shape == (N, 2) and directions.shape == (N, 2)
    assert latents.shape == (M, 6)
    origins = np.ascontiguousarray(origins, dtype=np.float32)
    directions = np.ascontiguousarray(directions, dtype=np.float32)
    latents = np.ascontiguousarray(latents, dtype=np.float32)

    if _CACHED_NC is None:
        _CACHED_NC = build_nc()
    nc = _CACHED_NC

    in_maps = []
    for c in range(N_CORES):
        sl = slice(c * N_LOC, (c + 1) * N_LOC)
        in_maps.append(
            {
                "origins": origins[sl],
                "directions": directions[sl],
                "latents": latents,
            }
        )

    results = bass_utils.run_bass_kernel_spmd(
        nc,
        in_maps,
        core_ids=list(range(N_CORES)),
        trace=TRACE,
    )
    LAST_PERF = results
    out = np.concatenate([results.results[c]["prob"] for c in range(N_CORES)], axis=0)
    return out.astype(np.float32)


if __name__ == "__main__":
    rng = np.random.default_rng(0)
    o = rng.standard_normal((N, 2), dtype=np.float32)
    d = rng.standard_normal((N, 2), dtype=np.float32)
    l = rng.standard_normal((M, 6), dtype=np.float32)
    p = kernel(o, d, l)
    print(p.shape, p.dtype, p.min(), p.max())


# revision 32
# speedup vs baseline: 1.1774x; 1.1053x over previous
"""Trainium2 Bass kernel for nn_Decoder_24541443129406.

Math: the reference's pdf/pdf_max cancels the normalization, so

    prob[n] = clip( sum_m exp( -0.5 * sum_d (pos[n,d]-mean[m,d])^2 / sigma[m,d] ), 0, 1 )

with pos = [ox, oy, dx, dy], sigma = [sx, sy, 1e-3, 1e-3],
sx = relu(l4)+0.01, sy = relu(l5)+0.01, mean = latents[:, :4].

The exponent is a quadratic form -> a K=8 matmul:
    e[n,m] = f[n] . w[m]
    f[n] = [dx^2+dy^2, 1, ox, oy, dx, dy, ox^2, oy^2]
    w[m] = [c7, c0, c1, c2, c3, c4, c5, c6]
      c1 = mx/sx, c2 = my/sy, c3 = 1000*mdx, c4 = 1000*mdy,
      c5 = -0.5/sx, c6 = -0.5/sy, c7 = -500,
      c0 = -0.5*(mx^2/sx + my^2/sy + 1000*(mdx^2+mdy^2))

fp32 matmuls are 4 cycles/row on the PE and float32r truncates, so the
K=8 fp32 matmul is emulated as one K=24 fp16 matmul with hi/lo split
operands stacked along K: e = h.H + l.H + h.L  (features f = h + l,
weights w = H + L, each half fp16; fp16 x fp16 products are exact in
fp32) — ~2^-22 relative accuracy at 1 cycle/row.

Per core (8 cores, data-parallel over rays): N_loc = 8192 rays, M = 512
gaussians. 16 super-tiles of 4 ray-blocks: 4x PE matmul -> PSUM
[128, 2048]; one ACT Exp pass -> fp16 [128, 2048] SBUF; per-block DVE
tensor_scalar with accum_out -> per-ray sums; clip; PE-transpose;
contiguous DMA out.
"""

import os
import sys
from contextlib import ExitStack

import numpy as np

for _p in ("/opt/trn_rl_repo", "/root/.axon_site/_ro/trn_rl_repo"):
    if os.path.isdir(_p) and _p not in sys.path:
        sys.path.insert(0, _p)

import concourse.bacc as bacc
import concourse.bass as bass
import concourse.mybir as mybir
import concourse.tile as tile
from concourse import bass_utils
from concourse.masks import make_identity

N_CORES = 8
N = 65536
M = 512
N_LOC = N // N_CORES  # 8192
NCHUNK = 32  # feature-build chunks (32-partition groups: verifier requires
# compute-op SBUF APs to start at partition 0/32/64/96)
CHUNK = N_LOC // NCHUNK  # 256
NBLK = N_LOC // 128  # 64 ray blocks of 128
NSUP = NBLK // 4  # 16 super-tiles of 4 blocks
SIGMA_EPS = 0.01
INV_SDIR = 1000.0  # 1/sigma_dir

F32 = mybir.dt.float32
F16 = mybir.dt.float16
ALU = mybir.AluOpType
ACTF = mybir.ActivationFunctionType

TRACE = False
LAST_PERF = None
_CACHED_NC = None


def build_kernel_body(nc, origins, directions, latents, prob):
    """origins/directions: [N_LOC, 2] f32 DRAM APs; latents [M, 6]; prob [N_LOC, 1]."""
    with tile.TileContext(nc) as tc, ExitStack() as ctx:
        singles = ctx.enter_context(tc.tile_pool(name="singles", bufs=1))
        scratch = ctx.enter_context(tc.tile_pool(name="scratch", bufs=4))

        # ---------------- input loads (parallel DMA queues) ----------------
        # Contiguous loads only; strided extraction happens on-chip where the
        # address generators make it free.
        raw_og = singles.tile([NCHUNK, 2 * CHUNK], F32)
        raw_dr = singles.tile([NCHUNK, 2 * CHUNK], F32)
        og_v = origins.rearrange("(i r) c -> i (r c)", i=NCHUNK)
        dr_v = directions.rearrange("(i r) c -> i (r c)", i=NCHUNK)
        # latents first: the weight path is the longest dependency chain
        lat32 = singles.tile([32, 96], F32)
        nc.scalar.dma_start(
            out=lat32, in_=latents.rearrange("(p j) f -> p (j f)", p=32)
        )
        nc.sync.dma_start(out=raw_dr[0:8, :], in_=dr_v[0:8, :])
        nc.gpsimd.dma_start(out=raw_dr[8:16, :], in_=dr_v[8:16, :])
        nc.scalar.dma_start(out=raw_dr[16:24, :], in_=dr_v[16:24, :])
        nc.sync.dma_start(out=raw_dr[24:32, :], in_=dr_v[24:32, :])
        nc.gpsimd.dma_start(out=raw_og[0:8, :], in_=og_v[0:8, :])
        nc.scalar.dma_start(out=raw_og[8:16, :], in_=og_v[8:16, :])
        nc.sync.dma_start(out=raw_og[16:24, :], in_=og_v[16:24, :])
        nc.gpsimd.dma_start(out=raw_og[24:32, :], in_=og_v[24:32, :])

        # ---------------- gaussian weights ----------------
        # Entire weight prep runs 32-lane in the lat32 [32, 16-latents x 6]
        # layout (x/y live in the free dim -> no cross-partition hops).
        # lat32 view: value (p, j, f) = feature f of latent m = 16p + j.
        latv = lat32.rearrange("p (j f) -> p j f", f=6)

        def wtile(name):
            return singles.tile([32, 16], F32, name=name, tag=name)

        sx = wtile("sx")
        sy = wtile("sy")
        nc.vector.tensor_scalar(
            out=sx, in0=latv[:, :, 4], scalar1=0.0, scalar2=SIGMA_EPS,
            op0=ALU.max, op1=ALU.add,
        )
        nc.vector.tensor_scalar(
            out=sy, in0=latv[:, :, 5], scalar1=0.0, scalar2=SIGMA_EPS,
            op0=ALU.max, op1=ALU.add,
        )
        rx, ry = wtile("rx"), wtile("ry")
        rscr = wtile("rscr")
        nc.vector.reciprocal_approx_accurate(out=rx, in_=sx, scratch=rscr)
        nc.vector.reciprocal_approx_accurate(out=ry, in_=sy, scratch=rscr)

        c1, c2, c3, c4, c5, c6 = (wtile(f"c{i}") for i in range(1, 7))
        nc.vector.tensor_mul(out=c1, in0=latv[:, :, 0], in1=rx)
        nc.vector.tensor_mul(out=c2, in0=latv[:, :, 1], in1=ry)
        nc.vector.tensor_scalar_mul(out=c3, in0=latv[:, :, 2], scalar1=INV_SDIR)
        nc.vector.tensor_scalar_mul(out=c4, in0=latv[:, :, 3], scalar1=INV_SDIR)
        nc.vector.tensor_scalar_mul(out=c5, in0=rx, scalar1=-0.5)
        nc.vector.tensor_scalar_mul(out=c6, in0=ry, scalar1=-0.5)
        # c0 = -0.5*mx*c1 - 0.5*my*c2 - 500*mdx^2 - 500*mdy^2
        qx, qy, qdx, qdy = wtile("qx"), wtile("qy"), wtile("qdx"), wtile("qdy")
        nc.vector.scalar_tensor_tensor(
            out=qx, in0=latv[:, :, 0], scalar=-0.5, in1=c1,
            op0=ALU.mult, op1=ALU.mult,
        )
        nc.vector.scalar_tensor_tensor(
            out=qy, in0=latv[:, :, 1], scalar=-0.5, in1=c2,
            op0=ALU.mult, op1=ALU.mult,
        )
        nc.vector.scalar_tensor_tensor(
            out=qdx, in0=latv[:, :, 2], scalar=-0.5 * INV_SDIR, in1=latv[:, :, 2],
            op0=ALU.mult, op1=ALU.mult,
        )
        nc.vector.scalar_tensor_tensor(
            out=qdy, in0=latv[:, :, 3], scalar=-0.5 * INV_SDIR, in1=latv[:, :, 3],
            op0=ALU.mult, op1=ALU.mult,
        )
        c0 = wtile("c0")
        nc.vector.tensor_add(out=qx, in0=qx, in1=qy)
        nc.vector.tensor_add(out=qdx, in0=qdx, in1=qdy)
        nc.vector.tensor_add(out=c0, in0=qx, in1=qdx)

        # fp16 hi/lo split + assembly into the stacked weight tile
        # wgs [24, M]: rows 0-7 = H, 8-15 = H, 16-23 = L
        # (row order in each group: 0=c7, 1=c0, 2=c1 ... 7=c6).
        # [32, 16] f16 partition-major stream == m-order [1, 512] row.
        wgs = singles.tile([24, M], F16)
        c7h = singles.tile([1, M], F16)
        c7l = singles.tile([1, M], F16)
        nc.vector.memset(c7h, -0.5 * INV_SDIR)
        nc.vector.memset(c7l, 0.0)
        nc.sync.dma_start(out=wgs[0:1, :], in_=c7h)
        nc.scalar.dma_start(out=wgs[8:9, :], in_=c7h)
        nc.gpsimd.dma_start(out=wgs[16:17, :], in_=c7l)

        eng3 = [nc.sync, nc.scalar, nc.gpsimd]
        for r, piece in enumerate([c0, c1, c2, c3, c4, c5, c6], start=1):
            h = singles.tile([32, 16], F16, name=f"wh{r}", tag=f"wh{r}")
            lo = singles.tile([32, 16], F16, name=f"wl{r}", tag=f"wl{r}")
            nc.vector.tensor_copy(out=h, in_=piece)
            nc.vector.tensor_tensor(out=lo, in0=piece, in1=h, op=ALU.subtract)
            eng3[r % 3].dma_start(out=wgs[r : r + 1, :], in_=h)
            eng3[(r + 1) % 3].dma_start(out=wgs[8 + r : 9 + r, :], in_=h)
            eng3[(r + 2) % 3].dma_start(out=wgs[16 + r : 17 + r, :], in_=lo)

        # ---------------- feature tiles ----------------
        ox, oy = raw_og[:, 0::2], raw_og[:, 1::2]
        dx, dy = raw_dr[:, 0::2], raw_dr[:, 1::2]

        # Two f-major tiles, 32-partition feature groups.
        # featA groups: 0=dx^2+dy^2, 1=ones, 2=ox, 3=oy
        # featB groups: 4=dx, 5=dy, 6=ox^2, 7=oy^2
        featA = singles.tile([128, CHUNK], F32)
        featB = singles.tile([128, CHUNK], F32)
        s1 = singles.tile([NCHUNK, CHUNK], F32)

        nc.vector.tensor_mul(out=featA[0:32, :], in0=dx, in1=dx)
        nc.vector.tensor_mul(out=s1, in0=dy, in1=dy)
        nc.vector.tensor_add(out=featA[0:32, :], in0=featA[0:32, :], in1=s1)
        nc.vector.memset(featA[32:64, :], 1.0)
        nc.scalar.copy(out=featA[64:96, :], in_=ox)
        nc.scalar.copy(out=featA[96:128, :], in_=oy)
        nc.gpsimd.tensor_copy(out=featB[0:32, :], in_=dx)
        nc.gpsimd.tensor_copy(out=featB[32:64, :], in_=dy)
        nc.vector.tensor_mul(out=featB[64:96, :], in0=ox, in1=ox)
        nc.vector.tensor_mul(out=featB[96:128, :], in0=oy, in1=oy)

        # fp16 hi/lo split of the features
        hA = singles.tile([128, CHUNK], F16)
        lA = singles.tile([128, CHUNK], F16)
        hB = singles.tile([128, CHUNK], F16)
        lB = singles.tile([128, CHUNK], F16)
        nc.vector.tensor_copy(out=hA, in_=featA)
        nc.vector.tensor_tensor(out=lA, in0=featA, in1=hA, op=ALU.subtract)
        nc.vector.tensor_copy(out=hB, in_=featB)
        nc.gpsimd.tensor_tensor(out=lB, in0=featB, in1=hB, op=ALU.subtract)

        # permute to featcs [24, N_LOC] fp16: rows 0-7 = h, 8-15 = l,
        # 16-23 = h (K-stack [h; l; h] paired with wgs = [H; H; L]:
        # e = h.H + l.H + h.L).  One DMA per feature group: [32, CHUNK]
        # partition-major stream == C-order [1, N_LOC] row.
        featcs = singles.tile([24, N_LOC], F16)
        eng = [nc.sync, nc.scalar, nc.gpsimd, nc.sync]
        for f in range(4):
            hsrcA = hA[32 * f : 32 * (f + 1), :]
            hsrcB = hB[32 * f : 32 * (f + 1), :]
            eng[f].dma_start(out=featcs[f : f + 1, :], in_=hsrcA)
            eng[f].dma_start(out=featcs[4 + f : 5 + f, :], in_=hsrcB)
            eng[f].dma_start(out=featcs[16 + f : 17 + f, :], in_=hsrcA)
            eng[f].dma_start(out=featcs[20 + f : 21 + f, :], in_=hsrcB)
            eng[(f + 1) % 3].dma_start(
                out=featcs[8 + f : 9 + f, :], in_=lA[32 * f : 32 * (f + 1), :]
            )
            eng[(f + 2) % 3].dma_start(
                out=featcs[12 + f : 13 + f, :], in_=lB[32 * f : 32 * (f + 1), :]
            )

        # identity for the output transpose (one-time, overlaps setup)
        ident = singles.tile([128, 128], F32)
        make_identity(nc, ident)

        # ---------------- main loop ----------------
        res = singles.tile([128, NBLK], F32)  # res[p, b] = sum_m exp(e), ray 128b+p
        with tc.tile_pool(name="psum", bufs=2, space="PSUM") as psum:
            for s in range(NSUP):
                ps = psum.tile([128, 4 * M], F32, tag="ps")
                for j in range(4):
                    b = 4 * s + j
                    nc.tensor.matmul(
                        out=ps[:, M * j : M * (j + 1)],
                        lhsT=featcs[:, 128 * b : 128 * (b + 1)],
                        rhs=wgs,
                        start=True,
                        stop=True,
                    )
                ex = scratch.tile([128, 4 * M], F16, tag="ex")
                nc.scalar.activation(out=ex, in_=ps, func=ACTF.Exp)
                for j in range(4):
                    b = 4 * s + j
                    # fold the two 256-wide halves and accumulate in one op
                    # (same InstTensorScalarPtr family as tensor_scalar+accum,
                    # which is HW-proven; fp16 in/out for the 2x packed mode)
                    sc = scratch.tile([128, M // 2], F16, tag="sc")
                    nc.vector.scalar_tensor_tensor(
                        out=sc,
                        in0=ex[:, M * j : M * j + M // 2],
                        scalar=0.0,
                        in1=ex[:, M * j + M // 2 : M * (j + 1)],
                        op0=ALU.add,
                        op1=ALU.add,
                        accum_out=res[:, b : b + 1],
                    )

        # clip to [0, 1]
        nc.vector.tensor_scalar(
            out=res, in0=res, scalar1=0.0, scalar2=1.0, op0=ALU.max, op1=ALU.min
        )

        # transpose [128, NBLK] -> [NBLK, 128] so DRAM writes are contiguous
        with tc.tile_pool(name="psumt", bufs=1, space="PSUM") as psumt:
            pst = psumt.tile([NBLK, 128], F32)
            nc.tensor.transpose(out=pst, in_=res[:, :], identity=ident[:, :])
            rest = singles.tile([NBLK, 128], F32)
            nc.vector.tensor_copy(out=rest, in_=pst)
            nc.sync.dma_start(
                out=prob.rearrange("(b p) o -> b (p o)", b=NBLK), in_=rest
            )


def build_nc():
    nc = bacc.Bacc("TRN2", target_bir_lowering=False, debug=False)
    origins = nc.dram_tensor("origins", [N_LOC, 2], F32, kind="ExternalInput").ap()
    directions = nc.dram_tensor("directions", [N_LOC, 2], F32, kind="ExternalInput").ap()
    latents = nc.dram_tensor("latents", [M, 6], F32, kind="ExternalInput").ap()
    prob = nc.dram_tensor("prob", [N_LOC, 1], F32, kind="ExternalOutput").ap()
    build_kernel_body(nc, origins, directions, latents, prob)
    nc.compile()
    return nc


def kernel(origins: np.ndarray, directions: np.ndarray, latents: np.ndarray) -> np.ndarray:
    global _CACHED_NC, LAST_PERF
    assert origins.shape == (N, 2) and directions.shape == (N, 2)
    assert latents.shape == (M, 6)
    origins = np.ascontiguousarray(origins, dtype=np.float32)
    directions = np.ascontiguousarray(directions, dtype=np.float32)
    latents = np.ascontiguousarray(latents, dtype=np.float32)

    if _CACHED_NC is None:
        _CACHED_NC = build_nc()
    nc = _CACHED_NC

    in_maps = []
    for c in range(N_CORES):
        sl = slice(c * N_LOC, (c + 1) * N_LOC)
        in_maps.append(
            {
                "origins": origins[sl],
                "directions": directions[sl],
                "latents": latents,
            }
        )

    results = bass_utils.run_bass_kernel_spmd(
        nc,
        in_maps,
        core_ids=list(range(N_CORES)),
        trace=TRACE,
    )
    LAST_PERF = results
    out = np.concatenate([results.results[c]["prob"] for c in range(N_CORES)], axis=0)
    return out.astype(np.float32)


if __name__ == "__main__":
    rng = np.random.default_rng(0)
    o = rng.standard_normal((N, 2), dtype=np.float32)
    d = rng.standard_normal((N, 2), dtype=np.float32)
    l = rng.standard_normal((M, 6), dtype=np.float32)
    p = kernel(o, d, l)
    print(p.shape, p.dtype, p.min(), p.max())
